# revision 3
# baseline (speedup 1.0000x reference)
"""Trainium2 Bass kernel v2 for nn_Decoder_46531675685089.

Layout: sequence-parallel (each core owns 128 s-positions x 2 batches =
256 tokens) with Ulysses-style AllToAll attention:
  - q/k/v projected locally (full weights) for own tokens; q/k emerge
    feature-major via stationary-weight-block matmuls, v token-major via
    stationary-activation matmuls.
  - One combined A2A redistributes (q,k,v) so each core holds one head
    PAIR over the full sequence; attention runs exactly like the old TP
    kernel (keys on partitions, ones-row denominator trick).
  - A second A2A returns attention outputs; out-proj (full Wo),
    residual, BatchNorm all run locally on own tokens (BN stats per
    s-position over (b, d) are fully local).
  - FFN fully local (full W1/W2 streamed in eighths).
Collectives per layer: 4 small AllToAlls (vs 3 RS + 3 AG before).
v biases are folded into bo on the host (bo_eff = bo + bv @ Wo).
"""

import numpy as np
import ml_dtypes

import concourse.bass as bass
import concourse.mybir as mybir
import concourse.tile as tile
from concourse.tile_rust import add_dep_helper
from concourse import bacc
from concourse.bass_utils import run_bass_kernel_spmd
from concourse.masks import make_identity

F32 = mybir.dt.float32
BF16 = mybir.dt.bfloat16
NPBF16 = ml_dtypes.bfloat16

R = 8            # cores
L = 4            # layers
B = 2            # batch
S = 1024         # sequence
D = 1024         # model dim
H = 16           # heads
DK = 64
F = 4096
CH = 128         # s positions per core
TOK = B * CH     # 256 own tokens, order (b, sl)
NT = B * S       # 2048 tokens, order (r, b, sl)
MASK = 512
EPS = 1e-5

AluOp = mybir.AluOpType
Act = mybir.ActivationFunctionType


# ---------------------------------------------------------------- builder --

def build_kernel(nc):
    t_in = {}

    def ein(name, shape, dt):
        t_in[name] = nc.dram_tensor(name, list(shape), dt, kind="ExternalInput")
        return t_in[name]

    x_chunk = ein("x_chunk", (B, CH, D), F32)
    xTc = ein("xTc", (8, 128, TOK), BF16)            # own-chunk FM (jd, p, (b sl))
    wq = {i: ein(f"wq{i}", (L, 8, 128, D), BF16) for i in (1, 2)}
    wk = {i: ein(f"wk{i}", (L, 8, 128, D), BF16) for i in (1, 2)}
    wv = {i: ein(f"wv{i}", (L, 8, 128, D), BF16) for i in (1, 2)}
    wo = {i: ein(f"wo{i}", (L, 8, 128, D), BF16) for i in (1, 2)}
    bqk = {i: ein(f"bqk{i}", (L, 2, 8, 128), F32) for i in (1, 2)}
    w1 = ein("w1", (L, 8, 128, F), BF16)
    w2 = ein("w2", (L, 32, 128, D), BF16)
    bf1 = ein("bf1", (L, 32, 128), F32)
    bias_bc = ein("bias_bc", (L, 3, D), F32)         # bo1_eff, bo2_eff, bf2
    gbe = ein("gbe", (L, 3, 2, CH), F32)

    out_chunk = nc.dram_tensor("out_chunk", [B, CH, D], F32, kind="ExternalOutput")

    # A2A exchange buffers, one set per (stage-parity, batch)
    qk_in, qk_out, v_in, v_out, a_in, a_out = [], [], [], [], [], []
    for p in range(4):
        qk_in.append(nc.dram_tensor(f"qk_in{p}", [R * 256, CH], BF16))
        qk_out.append(nc.dram_tensor(f"qk_out{p}", [R * 256, CH], BF16))
        v_in.append(nc.dram_tensor(f"v_in{p}", [R * 128, CH], BF16))
        v_out.append(nc.dram_tensor(f"v_out{p}", [R * 128, CH], BF16))
        a_in.append(nc.dram_tensor(f"a_in{p}", [R * 128, CH], BF16))
        a_out.append(nc.dram_tensor(f"a_out{p}", [R * 128, CH], BF16))

    groups = [list(range(R))]

    with tile.TileContext(nc) as tc:
        import contextlib
        ctx = contextlib.ExitStack()
        with ctx:
            consts = ctx.enter_context(tc.tile_pool(name="consts", bufs=1))
            wqkv_p = ctx.enter_context(tc.tile_pool(name="wqkv", bufs=2))
            wo_p = ctx.enter_context(tc.tile_pool(name="wop", bufs=1))
            wf_p = ctx.enter_context(tc.tile_pool(name="wfp", bufs=2))
            proj_p = ctx.enter_context(tc.tile_pool(name="projp", bufs=1))
            seq_p = ctx.enter_context(tc.tile_pool(name="seqp", bufs=2))
            attn_p = ctx.enter_context(tc.tile_pool(name="attnp", bufs=2))
            expT_p = ctx.enter_context(tc.tile_pool(name="expTp", bufs=3))
            aT_p = ctx.enter_context(tc.tile_pool(name="aTp", bufs=1))
            rd_p = ctx.enter_context(tc.tile_pool(name="rdp", bufs=2))
            chunk_p = ctx.enter_context(tc.tile_pool(name="chunkp", bufs=1))
            stat_p = ctx.enter_context(tc.tile_pool(name="statp", bufs=4))
            res_p = ctx.enter_context(tc.tile_pool(name="resp", bufs=3))
            xT_p = ctx.enter_context(tc.tile_pool(name="xTp", bufs=3))
            hid_p = ctx.enter_context(tc.tile_pool(name="hidp", bufs=1))
            dram_p = ctx.enter_context(tc.tile_pool(name="dramp", bufs=4, space="DRAM"))
            ps_mm = ctx.enter_context(tc.tile_pool(name="ps_mm", bufs=4, space="PSUM"))
            ps_acc = ctx.enter_context(tc.tile_pool(name="ps_acc", bufs=4, space="PSUM"))

            last_cc = [None]

            def chained_cc(*args, **kw):
                cc = nc.gpsimd.collective_compute(*args, **kw)
                ci = cc.ins if hasattr(cc, "ins") else cc
                if last_cc[0] is not None:
                    add_dep_helper(ci, last_cc[0], sync=True,
                                   reason="serialize collectives")
                last_cc[0] = ci
                return cc

            ident32 = consts.tile([128, 128], F32)
            make_identity(nc, ident32)
            eps_t = consts.tile([128, 1], F32)
            nc.vector.memset(eps_t, EPS)

            # ---------- helpers ----------
            def bn_normalize(li, si, u, keep_tag):
                """BatchNorm over (b, d) per s-row of u [128, B, D] fp32.
                Returns normalized fp32 tile from res pool."""
                g_sb = stat_p.tile([128, 2], F32, tag="gbe")
                nc.sync.dma_start(out=g_sb,
                                  in_=gbe.ap()[li, si].rearrange("n s -> s n"))
                stats = stat_p.tile([128, 2 * B, 6], F32, tag="bnst")
                for b in range(B):
                    for half in range(2):
                        nc.vector.bn_stats(stats[:, 2 * b + half],
                                           u[:, b, 512 * half:512 * half + 512])
                mv = stat_p.tile([128, 2], F32, tag="mv")
                nc.vector.bn_aggr(mv, stats)
                std = stat_p.tile([128, 1], F32, tag="std")
                nc.scalar.activation(std, mv[:, 1:2], Act.Sqrt, bias=eps_t)
                rstd = stat_p.tile([128, 1], F32, tag="rstd")
                nc.vector.reciprocal(rstd, std)
                A_t = stat_p.tile([128, 1], F32, tag="A")
                nc.vector.tensor_mul(A_t, rstd, g_sb[:, 0:1])
                mA = stat_p.tile([128, 1], F32, tag="mA")
                nc.vector.tensor_mul(mA, mv[:, 0:1], A_t)
                B_t = stat_p.tile([128, 1], F32, tag="Bt")
                nc.vector.tensor_sub(B_t, g_sb[:, 1:2], mA)
                xn = res_p.tile([128, B, D], F32, tag="res", name=keep_tag)
                for b in range(B):
                    nc.vector.tensor_scalar(xn[:, b], u[:, b], A_t, B_t,
                                            AluOp.mult, AluOp.add)
                return xn

            def to_fm(xn, name):
                """Transpose normalized chunk [128(sl), B, D] fp32 ->
                FM bf16 [128(dp), 8(jd), TOK] with cols (b, sl)."""
                xT = xT_p.tile([128, 8, TOK], BF16, tag="xT", name=name)
                for b in range(B):
                    for jd in range(8):
                        tp = ps_mm.tile([128, 128], F32, tag="mm", name="tp")
                        nc.tensor.transpose(tp, xn[:, b, 128 * jd:128 * jd + 128],
                                            ident32)
                        nc.vector.tensor_copy(xT[:, jd, 128 * b:128 * b + 128], tp)
                return xT

            def residual_bn(li, si, psums, res_in, keep_tag):
                """u[:,b] = psums[b] (+halves) + res_in[:,b] + bias row; BN."""
                bb = chunk_p.tile([128, D], F32, tag="bb")
                brow = bias_bc.ap()[li, si]
                nc.sync.dma_start(
                    out=bb,
                    in_=bass.AP(tensor=brow.tensor, offset=brow.offset,
                                ap=[[0, 128]] + brow.ap))
                u = chunk_p.tile([128, B, D], F32, tag="u")
                for b in range(B):
                    for nh in range(2):
                        sl = slice(512 * nh, 512 * nh + 512)
                        nc.vector.tensor_add(u[:, b, sl], psums[b][nh], bb[:, sl])
                    nc.vector.tensor_add(u[:, b], u[:, b], res_in[:, b])
                return bn_normalize(li, si, u, keep_tag)

            # ---------- attention stage ----------
            def attn_stage(li, i, xT_in, res_in):
                masked = (i == 1)
                KVR = 4 if masked else 8
                par = (li * 2 + (i - 1)) % 2

                wq_sb = wqkv_p.tile([128, 8, D], BF16, tag="wqkv", name="wq_sb")
                nc.scalar.dma_start(out=wq_sb, in_=wq[i].ap()[li].rearrange("j p c -> p j c"))
                wk_sb = wqkv_p.tile([128, 8, D], BF16, tag="wqkv", name="wk_sb")
                nc.scalar.dma_start(out=wk_sb, in_=wk[i].ap()[li].rearrange("j p c -> p j c"))
                wv_sb = wqkv_p.tile([128, 8, D], BF16, tag="wqkv", name="wv_sb")
                nc.scalar.dma_start(out=wv_sb, in_=wv[i].ap()[li].rearrange("j p c -> p j c"))
                wo_sb = wo_p.tile([128, 8, D], BF16, tag="wo")
                nc.scalar.dma_start(out=wo_sb, in_=wo[i].ap()[li].rearrange("j p c -> p j c"))
                bqk_sb = stat_p.tile([128, 2, 8], F32, tag="bqk")
                nc.sync.dma_start(out=bqk_sb,
                                  in_=bqk[i].ap()[li].rearrange("n f p -> p n f"))

                # q/k feature-major [128(hh dk), 8(hp), TOK]
                def qk_proj(w_sb, bi, nm):
                    o = proj_p.tile([128, 8, TOK], BF16, tag=nm, name=nm)
                    for fb in range(8):
                        ps = ps_mm.tile([128, TOK], F32, tag="mm", name=f"{nm}ps")
                        for jd in range(8):
                            nc.tensor.matmul(ps, w_sb[:, jd, 128 * fb:128 * fb + 128],
                                             xT_in[:, jd], start=(jd == 0),
                                             stop=(jd == 7))
                        nc.vector.tensor_scalar_add(o[:, fb], ps,
                                                    bqk_sb[:, bi, fb:fb + 1])
                    return o

                qins = [qk_in[2 * par + b].ap().rearrange("(j x) t -> j x t", x=256)
                        for b in range(B)]
                q_fm = qk_proj(wq_sb, 0, "qfm")
                for b in range(B):
                    nc.sync.dma_start(
                        out=qins[b][:, 0:128].rearrange("j p t -> p j t"),
                        in_=q_fm[:, :, 128 * b:128 * b + 128])
                k_fm = qk_proj(wk_sb, 1, "kfm")
                for b in range(B):
                    nc.sync.dma_start(
                        out=qins[b][:, 128:256].rearrange("j p t -> p j t"),
                        in_=k_fm[:, :, 128 * b:128 * b + 128])

                # v token-major [128(sl), B, 1024(h dk)]
                v16 = proj_p.tile([128, B, D], BF16, tag="v16", name="v16")
                for tb in range(B):
                    for nh in range(2):
                        ps = ps_mm.tile([128, 512], F32, tag="mm", name="vps")
                        for jd in range(8):
                            nc.tensor.matmul(
                                ps, xT_in[:, jd, 128 * tb:128 * tb + 128],
                                wv_sb[:, jd, 512 * nh:512 * nh + 512],
                                start=(jd == 0), stop=(jd == 7))
                        nc.vector.tensor_copy(v16[:, tb, 512 * nh:512 * nh + 512], ps)
                for b in range(B):
                    # v shard j: [128(sl), 128(dk2)] token-major
                    nc.sync.dma_start(
                        out=v_in[2 * par + b].ap().rearrange(
                            "(j p) t -> p j t", p=128),
                        in_=v16[:, b].rearrange("p (j d) -> p j d", j=8))
                    # chain order qk(b), v(b) so batch b's attention is
                    # never blocked behind the other batch's exchange
                    chained_cc(
                        "AllToAll", AluOp.bypass, replica_groups=groups,
                        ins=[qk_in[2 * par + b].ap()],
                        outs=[qk_out[2 * par + b].ap()])
                    chained_cc(
                        "AllToAll", AluOp.bypass, replica_groups=groups,
                        ins=[v_in[2 * par + b].ap()],
                        outs=[v_out[2 * par + b].ap()])

                # per-batch attention over my head pair; A2A(b1) hides
                # under attention(b0), return-A2A(b0) under attention(b1)
                for b in range(B):
                    qout = qk_out[2 * par + b].ap().rearrange(
                        "(j x) t -> j x t", x=256)
                    qT = seq_p.tile([128, 8, CH], BF16, tag="qT", name="qT")
                    nc.sync.dma_start(
                        out=qT, in_=qout[:, 0:128].rearrange("j p t -> p j t"))
                    kT = seq_p.tile([128, 8, CH], BF16, tag="kT", name="kT")
                    nc.sync.dma_start(
                        out=kT[:, 0:KVR],
                        in_=qout[0:KVR, 128:256].rearrange("j p t -> p j t"))
                    vout = v_out[2 * par + b].ap().rearrange(
                        "(j p) t -> j p t", p=128)
                    vaug = attn_p.tile([128, KVR, 2, 65], BF16, tag="vaug")
                    nc.vector.memset(vaug, 1.0)
                    for kt in range(KVR):
                        nc.sync.dma_start(
                            out=vaug[:, kt, :, 0:64],
                            in_=vout[kt].rearrange("t (h d) -> t h d", h=2))
                    q4, k4 = qT, kT
                    aT_h = [aT_p.tile([64, S], BF16, tag=f"aT{h}", name=f"aT{h}{b}")
                            for h in range(2)]
                    for n in range(2):
                        # both heads' score matmuls pack into concurrent
                        # row-groups (K=64 at base partitions 0 / 64)
                        expTs = [expT_p.tile([128, KVR, 512], BF16, tag="expT",
                                                name=f"expT{_h}")
                                 for _h in range(2)]
                        for kt in range(KVR):
                            for hh in range(2):
                                hp = slice(64 * hh, 64 * hh + 64)
                                sc = ps_mm.tile([128, 512], F32, tag="mm", name="sc")
                                nc.tensor.matmul(
                                    sc, k4[hp, kt], q4[hp, 4 * n:4 * n + 4],
                                    start=True, stop=True)
                                nc.scalar.activation(expTs[hh][:, kt], sc, Act.Exp,
                                                     scale=1.0 / np.sqrt(DK))
                        for hh in range(2):
                            expT = expTs[hh]
                            av = ps_acc.tile([65, 512], F32, tag="av", name="av")
                            for kt in range(KVR):
                                nc.tensor.matmul(av, vaug[:, kt, hh], expT[:, kt],
                                                 start=(kt == 0), stop=(kt == KVR - 1))
                            avsb = attn_p.tile([65, 512], F32, tag="avsb")
                            nc.scalar.copy(avsb, av)
                            rec = rd_p.tile([1, 512], F32, tag="rec")
                            den = rd_p.tile([1, 512], F32, tag="den")
                            nc.sync.dma_start(out=den, in_=avsb[64:65, :])
                            nc.vector.reciprocal_approx_fast(rec, den)
                            rec16 = rd_p.tile([1, 512], BF16, tag="rec16")
                            nc.vector.tensor_copy(rec16, rec)
                            rdram = dram_p.tile([1, 512], BF16, tag="rdram")
                            nc.scalar.dma_start(out=rdram, in_=rec16)
                            bcs = attn_p.tile([64, 512], BF16, tag="bcs")
                            rrow = rdram[0]
                            nc.scalar.dma_start(
                                out=bcs,
                                in_=bass.AP(tensor=rrow.tensor, offset=rrow.offset,
                                            ap=[[0, 64]] + rrow.ap))
                            ah4 = aT_h[hh].rearrange("p (r s) -> p r s", s=CH)
                            nc.vector.tensor_mul(ah4[:, 4 * n:4 * n + 4],
                                                 avsb[0:64, :], bcs)
                    adst = a_in[2 * par + b].ap().rearrange(
                        "(j hh d) t -> hh d j t", hh=2, d=64)
                    for hh in range(2):
                        nc.sync.dma_start(
                            out=adst[hh],
                            in_=aT_h[hh].rearrange("d (j t) -> d j t", t=CH))
                    chained_cc(
                        "AllToAll", AluOp.bypass, replica_groups=groups,
                        ins=[a_in[2 * par + b].ap()], outs=[a_out[2 * par + b].ap()])

                # out-proj (full Wo) per batch + residual
                bb = chunk_p.tile([128, D], F32, tag="bb")
                brow = bias_bc.ap()[li, i - 1]
                nc.sync.dma_start(
                    out=bb,
                    in_=bass.AP(tensor=brow.tensor, offset=brow.offset,
                                ap=[[0, 128]] + brow.ap))
                u = chunk_p.tile([128, B, D], F32, tag="u")
                for b in range(B):
                    a_fm = seq_p.tile([128, 8, CH], BF16, tag="afm", name="afm")
                    nc.sync.dma_start(
                        out=a_fm,
                        in_=a_out[2 * par + b].ap().rearrange(
                            "(fb p) t -> p fb t", p=128))
                    pss = [ps_acc.tile([128, 512], F32, tag="av", name=f"op{b}{nh}")
                           for nh in range(2)]
                    for fb in range(8):
                        for nh in range(2):
                            nc.tensor.matmul(
                                pss[nh], a_fm[:, fb],
                                wo_sb[:, fb, 512 * nh:512 * nh + 512],
                                start=(fb == 0), stop=(fb == 7))
                    for nh in range(2):
                        sl = slice(512 * nh, 512 * nh + 512)
                        nc.vector.tensor_add(u[:, b, sl], pss[nh], bb[:, sl])
                    nc.vector.tensor_add(u[:, b], u[:, b], res_in[:, b])
                xn = bn_normalize(li, i - 1, u, f"xn{li}{i}")
                xT_out = to_fm(xn, f"xT{li}{i}")
                return xn, xT_out

            # ---------- FFN stage ----------
            def ffn_stage(li, x1, yT):
                bf1_sb = stat_p.tile([128, 32], F32, tag="bf1")
                nc.sync.dma_start(out=bf1_sb,
                                  in_=bf1.ap()[li].rearrange("f p -> p f"))
                hid = hid_p.tile([128, 32, TOK], BF16, tag="hid")
                psums = [[ps_acc.tile([128, 512], F32, tag="av", name=f"f2{tb}{nh}")
                          for nh in range(2)] for tb in range(B)]

                def ffn1_chunk(e):
                    w1e = wf_p.tile([128, 8, 256], BF16, tag="w1e", name="w1e")
                    nc.scalar.dma_start(
                        out=w1e,
                        in_=w1.ap()[li, :, :, 256 * e:256 * e + 256]
                        .rearrange("j p c -> p j c"))
                    for fbl in range(2):
                        fb = 2 * e + fbl
                        ps = ps_mm.tile([128, TOK], F32, tag="mm", name="f1ps")
                        for jd in range(8):
                            nc.tensor.matmul(
                                ps, w1e[:, jd, 128 * fbl:128 * fbl + 128],
                                yT[:, jd], start=(jd == 0), stop=(jd == 7))
                        nc.scalar.activation(hid[:, fb], ps, Act.Relu,
                                             bias=bf1_sb[:, fb:fb + 1])

                def ffn2_chunk(e):
                    w2e = wf_p.tile([128, 2, D], BF16, tag="w2e", name="w2e")
                    nc.scalar.dma_start(
                        out=w2e,
                        in_=w2.ap()[li, 2 * e:2 * e + 2].rearrange("f p d -> p f d"))
                    for fbl in range(2):
                        fb = 2 * e + fbl
                        for tb in range(B):
                            for nh in range(2):
                                nc.tensor.matmul(
                                    psums[tb][nh],
                                    hid[:, fb, 128 * tb:128 * tb + 128],
                                    w2e[:, fbl, 512 * nh:512 * nh + 512],
                                    start=(fb == 0), stop=(fb == 31))

                ffn1_chunk(0)
                for e in range(1, 16):
                    ffn1_chunk(e)
                    ffn2_chunk(e - 1)
                ffn2_chunk(15)
                xn = residual_bn(li, 2, psums, x1, f"h{li}")
                return xn

            # ---------- main ----------
            res = res_p.tile([128, B, D], F32, tag="res", name="h_init")
            nc.sync.dma_start(out=res, in_=x_chunk.ap().rearrange("b s d -> s b d"))
            hT = xT_p.tile([128, 8, TOK], BF16, tag="xT", name="hT_init")
            nc.sync.dma_start(out=hT, in_=xTc.ap().rearrange("j p t -> p j t"))

            for li in range(L):
                x1, x1T = attn_stage(li, 1, hT, res)
                _y, yT = attn_stage(li, 2, x1T, res)
                res = ffn_stage(li, x1, yT)
                if li < L - 1:
                    hT = to_fm(res, f"hT{li}")
                else:
                    nc.sync.dma_start(
                        out=out_chunk.ap().rearrange("b s d -> s b d"), in_=res)

    return nc


# ---------------------------------------------------------------- host ----

_CACHE = {}


def _get_compiled():
    if "nc" not in _CACHE:
        nc = bacc.Bacc("TRN2", target_bir_lowering=False, debug=False,
                       num_devices=R)
        build_kernel(nc)
        nc.compile()
        _CACHE["nc"] = nc
    return _CACHE["nc"]


def _prep_shared_inputs(inp):
    """Weight tensors identical on every core."""
    f32 = np.float32

    def bf(a):
        return np.ascontiguousarray(np.asarray(a, f32).astype(NPBF16))

    m = {}
    for i in (1, 2):
        for nm, w in (("wq", inp[f"Wq{i}"]), ("wk", inp[f"Wk{i}"]),
                      ("wv", inp[f"Wv{i}"])):
            wc = np.asarray(w, f32).transpose(0, 2, 1, 3).reshape(L, D, H * DK)
            m[f"{nm}{i}"] = bf(wc.reshape(L, 8, 128, H * DK))
        m[f"wo{i}"] = bf(np.asarray(inp[f"Wo{i}"], f32).reshape(L, 8, 128, D))
        bq = np.asarray(inp[f"bq{i}"], f32).reshape(L, 8, 128)
        bk = np.asarray(inp[f"bk{i}"], f32).reshape(L, 8, 128)
        m[f"bqk{i}"] = np.ascontiguousarray(np.stack([bq, bk], axis=1))
    m["w1"] = bf(np.asarray(inp["W1"], f32).reshape(L, 8, 128, F))
    m["w2"] = bf(np.asarray(inp["W2"], f32).reshape(L, 32, 128, D))
    m["bf1"] = np.ascontiguousarray(np.asarray(inp["bf1"], f32).reshape(L, 32, 128))
    bias = []
    for i in (1, 2):
        bv_flat = np.asarray(inp[f"bv{i}"], f32).reshape(L, H * DK)
        bo_eff = np.asarray(inp[f"bo{i}"], f32) + np.einsum(
            "lf,lfd->ld", bv_flat, np.asarray(inp[f"Wo{i}"], f32))
        bias.append(bo_eff.astype(f32))
    bias.append(np.asarray(inp["bf2"], f32))
    m["bias_bc"] = np.ascontiguousarray(np.stack(bias, axis=1))
    return m


def _prep_core_inputs(inp, c, shared):
    f32 = np.float32
    x = np.asarray(inp["x"], f32)
    m = dict(shared)
    xc = x[:, c * CH:(c + 1) * CH, :]                       # [B, 128, 1024]
    m["x_chunk"] = np.ascontiguousarray(xc)
    m["xTc"] = np.ascontiguousarray(
        xc.reshape(B, CH, 8, 128).transpose(2, 3, 0, 1)
        .reshape(8, 128, TOK).astype(NPBF16))
    sl = slice(CH * c, CH * (c + 1))
    m["gbe"] = np.ascontiguousarray(np.stack(
        [np.stack([np.asarray(inp[f"g{j}"], f32)[:, sl],
                   np.asarray(inp[f"be{j}"], f32)[:, sl]], axis=1)
         for j in (1, 2, 3)], axis=1))
    return m


def kernel(**inputs):
    nc = _get_compiled()
    shared = _prep_shared_inputs(inputs)
    in_maps = [_prep_core_inputs(inputs, c, shared) for c in range(R)]
    res = run_bass_kernel_spmd(nc, in_maps, list(range(R)))
    chunks = [res.results[c]["out_chunk"] for c in range(R)]
    out = np.concatenate(chunks, axis=1).astype(np.float32)
    return out
